# revision 91
# baseline (speedup 1.0000x reference)
"""Multi-head causal self-attention (B=2, T=2048, C=1024, H=16, D=64) on 8
Trainium2 NeuronCores.

Sharding: data-parallel over batch (2) x tensor-parallel over heads (4 groups
of 4 heads) = 8 shards, no cross-core communication. Host sums the 4 partial
outputs per batch and adds the (folded) bias.

Precision/PE strategy (the PE and Act engines are co-critical):
- qkv projection: fp8e4m3 DoubleRow with residual splitting. x and wqkv ship
  as (hi, lo) e4m3 pairs at power-of-2 scales (x:4, w:16); the A-term
  (hi*hi, contraction 256 per DR matmul) plus B-term (hi*lo + lo*hi cross
  residuals as the two DR k-tiles of one matmul) all carry a uniform 64x
  scale that the psum->sbuf copy divides out (fused mult+bias tensor_scalar).
  Accuracy is bf16-class at ~40% fewer PE cycles; the dropped lo*lo term is
  ~0.1% of signal.
- scores: q,k stored as e4m3 in a head-local layout (head h's dims at
  partitions 32h:32h+32, lo/hi halves as the two DR k-tiles via a host-side
  wqkv column permutation), one DoubleRow matmul per (key-block, head) at
  tile_position (32h, 0) = 0.5 PE cycles/query. Costs ~1.4e-2 rel err
  (gate 2e-2); fp8 for PV/proj fails the gate and stays bf16.
- exp: head-PAIRED - both heads of a pair write the two banks of one
  [128, 2, 512] psum tile, so one Act instruction covers both (halves the
  ~185ns/instr Act access overhead). ~1/4 of pair-exps are computed on the
  DVE with a bf16 Schraudolph (int16 bitcast of x*2^7/ln2 + 16249; max ~4%
  p error, shared bias cancels in the softmax num/den ratio), balancing the
  Act and DVE engines.
- PV keeps the ones-column trick (65th V column accumulates the softmax
  denominator per query partition) and runs qb-major so only 2 attps banks
  are live (PSUM: 2 filler + 4 score-pair + 2 attps banks = 8).
- attT via XBAR DMA transpose (tq 0-2; c-major 3D dest matches the attT
  layout) and PE transpose on the last tq's short tail chains; out =
  attT.T @ wproj in bf16.

Scheduling: explicit priority bands on the tile list-scheduler. Score/exp
chains keep default (lowest=first) priorities so the Act engine's feed is
never queued behind other PE work; band B = q/k filler units for the next
tq (they gate the NEXT exp stream); band C = v units and qb-major PV+norm
(v(tq+1) is emitted after PV(tq) so PV frees pt tiles - which gate the next
tq's exps via pool rotation - before any deferrable work); band D = all
projection, backloaded into the final (most exp-bound) blocks. Weight/x
DMAs land hi-slots first (A-terms open every psum group). The causal mask
zeroes the diagonal blocks' upper triangle in-place on the otherwise-idle
GPSIMD engine; all bias matmuls fold into the host epilogue (softmax rows
sum to 1, so the V bias contributes bv @ w_proj to every output row).
"""

from contextlib import contextmanager

import numpy as np
import ml_dtypes

import concourse.bass as bass
import concourse.mybir as mybir
import concourse.tile as tile
from concourse import bacc
from concourse.bass_utils import run_bass_kernel_spmd

f32 = mybir.dt.float32
bf16 = mybir.dt.bfloat16
f8e4 = mybir.dt.float8e4
DRMODE = mybir.MatmulPerfMode.DoubleRow
AF = mybir.ActivationFunctionType
ALU = mybir.AluOpType

B, T, C, H, D = 2, 2048, 1024, 16, 64
HPC = 4          # heads per core
NCORES = 8
TQ = 512         # query tile of the attention outer loop
NTQ = T // TQ    # 4
NKC = C // 128   # 8 contraction chunks for the qkv projection
NTT = T // 128   # 16 query 128-blocks
SCALE = 1.0 / 8.0  # 1/sqrt(D)

_CACHE = {}


def build_nc():
    nc = bacc.Bacc("TRN2", target_bir_lowering=False, debug=False)

    # x and wqkv ship as fp8 (hi, lo) residual pairs: hi = e4m3(a*t),
    # lo = e4m3(a*(t - hi/a)), with a=4 for x and a=16 for w. Every qkv
    # product term then carries a uniform 64x scale that the psum->sbuf
    # copy divides out, so qkv accuracy is bf16-class at fp8 DR speed.
    xt_d = nc.dram_tensor("xt8", [C, 2, T], f8e4, kind="ExternalInput")
    wqkv_d = nc.dram_tensor("wqkv8", [2, C, 768], f8e4, kind="ExternalInput")
    bqk_d = nc.dram_tensor("bqk", [128, 4], f32, kind="ExternalInput")
    wproj_d = nc.dram_tensor("wproj", [256, C], bf16, kind="ExternalInput")
    out_d = nc.dram_tensor("out", [T, C], bf16, kind="ExternalOutput")

    with tile.TileContext(nc) as tc:
        with (
            tc.tile_pool(name="const", bufs=1) as const,
            tc.tile_pool(name="xts", bufs=2) as xts_pool,
            tc.tile_pool(name="pt", bufs=65) as pt_pool,
            tc.tile_pool(name="atts", bufs=8) as atts_pool,
            tc.tile_pool(name="rec", bufs=8) as rec_pool,
            tc.tile_pool(name="ot", bufs=8) as ot_pool,
            tc.tile_pool(name="ps_mm", bufs=2, space="PSUM") as ps_mm,
            tc.tile_pool(name="ps_sc", bufs=2, space="PSUM") as ps_sc,
            tc.tile_pool(name="ps_att", bufs=2, space="PSUM") as ps_att,
        ):
            # Priority bands: the tile list-scheduler pops ready work by
            # ascending priority. Score/exp chains keep the default
            # (emission-order, lowest) priorities; later bands hold work
            # that must never delay the Act engine's feed chain.
            BAND_QKV, BAND_PV, BAND_PROJ = 1_000_000, 2_000_000, 3_000_000
            _band_next = {}

            @contextmanager
            def prio_band(band):
                saved = tc.cur_priority
                tc.cur_priority = _band_next.get(band, band)
                try:
                    yield
                finally:
                    _band_next[band] = tc.cur_priority
                    tc.cur_priority = saved
            # ---- resident tensors; DMAs chunked so compute starts early ----
            # wqkv slots (host order): 0 = lo, 1 = hi -- B-term k-tile pairs
            # (w_lo, w_hi) and A-term hi slices both slice positively.
            # x slots (host order): 0 = hi, 1 = lo -- B pairs (x_hi, x_lo).
            wqkv_sb = const.tile([128, NKC, 2, 768], f8e4, tag="wqkv")
            wqkv_r = wqkv_d.rearrange("s (o p) n -> p o s n", p=128)
            # hi slots (s=1) first: the A-term hi*hi matmuls open every
            # psum group; lo slots are only needed once B-terms start.
            nc.scalar.dma_start(wqkv_sb[:, 0:2, 1, :], wqkv_r[:, 0:2, 1, :])
            nc.scalar.dma_start(wqkv_sb[:, 2:NKC, 1, :], wqkv_r[:, 2:NKC, 1, :])
            nc.scalar.dma_start(wqkv_sb[:, 0:4, 0, :], wqkv_r[:, 0:4, 0, :])
            nc.scalar.dma_start(wqkv_sb[:, 4:NKC, 0, :], wqkv_r[:, 4:NKC, 0, :])
            bqk_sb = const.tile([128, 4], f32, tag="bqk")
            nc.scalar.dma_start(bqk_sb[:], bqk_d[:, :])
            wproj_sb = const.tile([128, 2, C], bf16, tag="wproj")
            nc.scalar.dma_start(wproj_sb[:], wproj_d.rearrange("(o p) n -> p o n", p=128))

            # qkT chunks (fp8, host-permuted): 0 = q lo-dims (4 heads x 32),
            # 1 = q hi-dims, 2 = k lo, 3 = k hi. Head h occupies partitions
            # 32h:32h+32; its (lo, hi) slots form the two DoubleRow k-tiles.
            qkT_sb = const.tile([128, 4, T], f8e4, tag="qkT")
            # v in PV-rhs layout: [key mod 128, key block, head, 64 vdims + one]
            v_sb = const.tile([128, NTT, HPC, 65], bf16, tag="v")
            nc.vector.memset(v_sb[:, :, :, 64:65], 1.0)
            # attT: chunk hc: partitions = head-dims of heads (2hc, 2hc+1)
            attT_sb = const.tile([128, 2, T], bf16, tag="attT")
            # identity for PE-transpose of the normalized attention
            ident_sb = const.tile([128, 128], bf16, tag="ident")
            nc.vector.memset(ident_sb[:], 1.0)
            nc.gpsimd.affine_select(
                ident_sb[:],
                ident_sb[:],
                pattern=[[1, 128]],
                compare_op=ALU.is_equal,
                fill=0.0,
                base=0,
                channel_multiplier=-1,
            )
            # lower-triangular causal mask (keep j >= p), applied to diagonal
            # blocks with a DVE multiply (lower latency than gpsimd select)
            tri_sb = const.tile([128, 128], bf16, tag="tri")
            nc.vector.memset(tri_sb[:], 1.0)
            nc.gpsimd.affine_select(
                tri_sb[:],
                tri_sb[:],
                pattern=[[1, 128]],
                compare_op=ALU.is_ge,
                fill=0.0,
                base=0,
                channel_multiplier=-1,
            )

            xt_r = xt_d.rearrange("(o p) s t -> p o s t", p=128)


            def qk_chunk_mms(ps, xts, cp, dma_aligned=False):
                """q/k chunk cp into psum ps: A-term (hi*hi, kc-paired DR)
                then B-term (hi*lo + lo*hi cross residuals, one DR per kc).
                All terms carry the uniform 64x host scale. dma_aligned
                orders terms by the kc of their LAST-arriving operand so the
                startup chunk never stalls on a not-yet-landed DMA piece."""
                c0 = 128 * cp

                def a_term(kcp):
                    return (
                        wqkv_sb[:, 2 * kcp : 2 * kcp + 2, 1, c0 : c0 + 128],
                        xts[:, 2 * kcp : 2 * kcp + 2, 0, :],
                        kcp == 0, False,
                    )

                def b_term(kc):
                    return (
                        wqkv_sb[:, kc, :, c0 : c0 + 128],
                        xts[:, kc, :, :],
                        False, kc == NKC - 1,
                    )

                if dma_aligned:
                    return (
                        [a_term(0)]
                        + [a_term(k) for k in range(1, NKC // 2)]
                        + [b_term(k) for k in range(NKC)]
                    )
                return [a_term(k) for k in range(NKC // 2)] + [
                    b_term(k) for k in range(NKC)
                ]

            def run_mms(ps, mms):
                for lhsT, rhs, start, stop in mms:
                    nc.tensor.matmul(
                        ps[:], lhsT=lhsT, rhs=rhs, perf_mode=DRMODE,
                        start=start, stop=stop, skip_group_check=True,
                    )

            def qk_copy(cp, tqs, ps):
                # qkT = psum/64 + bias, converting to the fp8 score layout
                nc.vector.tensor_scalar(
                    qkT_sb[:, cp, tqs], ps[:], 1.0 / 64.0,
                    bqk_sb[:, cp : cp + 1], op0=ALU.mult, op1=ALU.add,
                )

            def emit_qkv(tq):
                """Prologue: q,k chunks only (v units are deferred to band C
                of att(0) - they gate only PV, and the first exp gates Act)."""
                tqs = slice(TQ * tq, TQ * (tq + 1))
                xts = xts_pool.tile([128, NKC, 2, TQ], f8e4, tag="xts")
                nc.sync.dma_start(xts[:, 0:2, 0, :], xt_r[:, 0:2, 0, tqs])
                nc.sync.dma_start(xts[:, 2:NKC, 0, :], xt_r[:, 2:NKC, 0, tqs])
                nc.sync.dma_start(xts[:, 0:4, 1, :], xt_r[:, 0:4, 1, tqs])
                nc.sync.dma_start(xts[:, 4:NKC, 1, :], xt_r[:, 4:NKC, 1, tqs])
                # step-major over pairs of open psum groups to hide DMA ramp
                for cps in ((0, 2), (1, 3)):
                    ps_pair = [
                        ps_mm.tile([128, TQ], f32, tag="mm", name=f"qk{cp}")
                        for cp in cps
                    ]
                    chunk_mms = [qk_chunk_mms(ps_pair[i], xts, cp)
                                 for i, cp in enumerate(cps)]
                    for j in range(len(chunk_mms[0])):
                        for i in range(len(cps)):
                            lhsT, rhs, start, stop = chunk_mms[i][j]
                            nc.tensor.matmul(
                                ps_pair[i][:], lhsT=lhsT, rhs=rhs,
                                perf_mode=DRMODE, start=start, stop=stop,
                                skip_group_check=True,
                            )
                    for i, cp in enumerate(cps):
                        qk_copy(cp, tqs, ps_pair[i])
                return xts

            def v_units_for(tq, xts):
                """v-projection units for t-slice tq, band C: v gates only PV
                of the same tq, so it runs after PV(tq-1) has released its pt
                tiles (which gate the NEXT tq's exp stream)."""
                tqs = slice(TQ * tq, TQ * (tq + 1))

                def v_unit(tt):
                    def emit():
                        with prio_band(BAND_PV):
                            psv = ps_mm.tile([128, TQ], f32, tag="mm")
                            toff = 128 * tt - TQ * tq
                            for kcp in range(NKC // 2):
                                nc.tensor.matmul(
                                    psv[:, 0:256],
                                    lhsT=xts[:, 2 * kcp : 2 * kcp + 2, 0,
                                             toff : toff + 128],
                                    rhs=wqkv_sb[:, 2 * kcp : 2 * kcp + 2, 1,
                                                512:768],
                                    perf_mode=DRMODE,
                                    start=(kcp == 0), stop=False,
                                    skip_group_check=True,
                                )
                            for kc in range(NKC):
                                nc.tensor.matmul(
                                    psv[:, 0:256],
                                    lhsT=xts[:, kc, :, toff : toff + 128],
                                    rhs=wqkv_sb[:, kc, :, 512:768],
                                    perf_mode=DRMODE,
                                    start=False, stop=(kc == NKC - 1),
                                    skip_group_check=True,
                                )
                            nc.vector.tensor_scalar_mul(
                                v_sb[:, tt, :, 0:64], psv[:, 0:256], 1.0 / 64.0
                            )
                    return emit

                return [v_unit(tt) for tt in range(4 * tq, 4 * tq + 4)]

            def qkv_units(tq):
                """q/k projection for t-slice tq as filler closures in band B:
                they gate tq's exp stream, so they run right after the prior
                tq's scores and before any PV."""
                tqs = slice(TQ * tq, TQ * (tq + 1))
                xts = xts_pool.tile([128, NKC, 2, TQ], f8e4, tag="xts")

                def load():
                    for s in range(2):
                        nc.sync.dma_start(xts[:, 0:4, s, :], xt_r[:, 0:4, s, tqs])
                        nc.sync.dma_start(xts[:, 4:NKC, s, :], xt_r[:, 4:NKC, s, tqs])

                def qk_unit(cp):
                    def emit():
                        with prio_band(BAND_QKV):
                            ps = ps_mm.tile([128, TQ], f32, tag="mm")
                            run_mms(ps, qk_chunk_mms(ps, xts, cp))
                            qk_copy(cp, tqs, ps)
                    return emit

                return load, xts, [qk_unit(cp) for cp in range(4)]

            def proj_units_qb(tq, qb):
                tt = 4 * tq + qb
                ot = [None]

                def emit(nt):
                    with prio_band(BAND_PROJ):
                        ts_ = slice(128 * tt, 128 * (tt + 1))
                        ns = slice(512 * nt, 512 * (nt + 1))
                        pso = ps_mm.tile([128, TQ], f32, tag="mm")
                        for hc in range(2):
                            nc.tensor.matmul(
                                pso[:],
                                lhsT=attT_sb[:, hc, ts_],
                                rhs=wproj_sb[:, hc, ns],
                                start=(hc == 0),
                                stop=(hc == 1),
                            )
                        if nt == 0:
                            ot[0] = ot_pool.tile(
                                [128, 2, TQ], bf16, tag="ot", name=f"ot{tt}"
                            )
                        if tq == NTQ - 1:
                            # last block: Act is done with exps by now while
                            # the DVE still drains normalize chains - use Act
                            # for the tail's psum copies, and ship each half
                            # as soon as its copy lands (HWDGE is idle at the
                            # end; a merged DMA would serialize the tail)
                            nc.scalar.copy(ot[0][:, nt, :], pso[:])
                            nc.sync.dma_start(out_d[ts_, ns], ot[0][:, nt, :])
                        else:
                            nc.vector.tensor_copy(ot[0][:, nt, :], pso[:])
                            if nt == 1:
                                # one merged DMA per 128-row block (fewer DMAs
                                # = less serialization on the 1-slot HWDGE)
                                nc.sync.dma_start(out_d[ts_, :], ot[0][:])

                return [lambda: emit(0), lambda: emit(1)]

            def proj_units(tq):
                units = []
                for qb in range(4):
                    units.extend(proj_units_qb(tq, qb))
                return units

            def emit_att(tq, qk_fillers, v_pre, v_post, last=False):
                """Attention for tq: the score/exp stream is emitted at the
                default (highest) priority band so the Act engine is never
                starved; q/k fillers for the next tq go in band B (they gate
                the NEXT tq's exp stream); PV runs qb-major in band C (it
                gates only the projection) with the per-qb normalize chain
                right after its PV chain; v units bracket PV in band C
                (v(tq) before PV(tq), v(tq+1) after, so PV(tq) frees pt
                tiles before any deferrable work); proj units in band D."""
                ntk = 4 * tq + 4
                # steps are (tk, head-pair): both heads' DR score matmuls land
                # in one 2-bank psum tile so a single exp covers them.
                steps = [(tk, hp) for tk in range(ntk) for hp in range(2)]
                pts = {}

                def emit_sc(i):
                    tk, hp = steps[i]
                    d = tk - 4 * tq
                    q0 = 128 * d if d >= 0 else 0
                    w = TQ - q0
                    ks = slice(128 * tk, 128 * (tk + 1))
                    qs = slice(TQ * tq + q0, TQ * (tq + 1))
                    sc = ps_sc.tile([128, 2, TQ], f32, tag="sc")
                    for sl in range(2):
                        h = 2 * hp + sl
                        p0 = 32 * h
                        nc.tensor.matmul(
                            sc[:, sl, 0:w],
                            lhsT=qkT_sb[p0 : p0 + 32, 2:4, ks],
                            rhs=qkT_sb[p0 : p0 + 32, 0:2, qs],
                            perf_mode=DRMODE,
                            tile_position=(p0, 0),
                        )
                    pt = pt_pool.tile([128, 2, TQ], bf16, tag="pt")
                    if (tq < 3 or i < 16) and i % 4 == 1:
                        # Schraudolph exp on the DVE: bf16 bits of exp2(x) ~
                        # round(x*2^7/ln2 + (16256 - C)); C=7 balances the
                        # piecewise-linear error (max ~4%, mean-bias ~0.2%,
                        # and the softmax num/den ratio cancels shared bias).
                        # Offloads ~1/7 of the exp stream from the saturated
                        # Act engine to the half-idle DVE.
                        nc.vector.tensor_scalar(
                            pt[:, :, 0:w].bitcast(mybir.dt.int16),
                            sc[:, :, 0:w],
                            SCALE * 184.6650292,
                            16249.0,
                            op0=ALU.mult,
                            op1=ALU.add,
                        )
                    else:
                        nc.scalar.activation(
                            pt[:, :, 0:w], sc[:, :, 0:w], AF.Exp, scale=SCALE
                        )
                    if d >= 0:
                        # causal mask on the diagonal block: zero the upper
                        # triangle in-place on the (otherwise idle) GPSIMD
                        # engine, freeing the DVE for exp/norm chains
                        for sl in range(2):
                            nc.gpsimd.affine_select(
                                pt[:, sl, 0:128],
                                pt[:, sl, 0:128],
                                pattern=[[1, 128]],
                                compare_op=ALU.is_ge,
                                fill=0.0,
                                base=0,
                                channel_multiplier=-1,
                            )
                    pts[i] = pt

                def emit_pv_qb(qb, attp):
                    for tk in range(4 * tq + qb + 1):
                        d = tk - 4 * tq
                        q0 = 128 * d if d >= 0 else 0
                        qoff = 128 * qb - q0
                        for h in range(HPC):
                            nc.tensor.matmul(
                                attp[:, h, 0:65],
                                lhsT=pts[2 * tk + h // 2][:, h % 2, qoff : qoff + 128],
                                rhs=v_sb[:, tk, h, 0:65],
                                start=(tk == 0 and h == 0),
                                stop=(tk == 4 * tq + qb and h == HPC - 1),
                                skip_group_check=True,
                            )

                def emit_norm(qb, attp):
                    rec = rec_pool.tile([128, HPC], f32, tag="rec")
                    nc.vector.reciprocal_approx_fast(out=rec[:], in_=attp[:, :, 64])
                    att_sb = atts_pool.tile([128, HPC, 64], bf16, tag="atts")
                    nc.vector.tensor_mul(
                        att_sb[:],
                        attp[:, :, 0:64],
                        rec[:, :, None].broadcast_to([128, HPC, 64]),
                    )
                    qslice = slice(TQ * tq + 128 * qb, TQ * tq + 128 * (qb + 1))
                    if tq < NTQ - 1:
                        # XBAR DMA transpose (att [q, hd] -> attT [hd, q]; the
                        # 3D dest maps transposed row r to (chunk r//128,
                        # partition r%128) = exactly the attT layout). Frees
                        # PE/DVE cycles and keeps the ps_att rotation to
                        # attps tiles only, so PV qb-chains overlap; its
                        # ~2.2us latency rides band C/D slack. SP queue: Act's
                        # SEQ must not pay the DMA setup time.
                        nc.sync.dma_start_transpose(
                            attT_sb[:, :, qslice], att_sb[:]
                        )
                    else:
                        # last tq: PE transpose (~53ns) keeps the terminal
                        # norm->proj chain short
                        attTps = ps_att.tile([128, 2, 128], bf16, tag="att")
                        for hc in range(2):
                            nc.tensor.matmul(
                                attTps[:, hc, :],
                                lhsT=att_sb[:, 2 * hc : 2 * hc + 2, :],
                                rhs=ident_sb[:],
                                is_transpose=True,
                                start=(hc == 0),
                                stop=(hc == 1),
                                skip_group_check=True,
                            )
                        nc.vector.tensor_copy(attT_sb[:, :, qslice], attTps[:])

                # score/exp stream at top priority: Act is the near-critical
                # engine, so its feed chain (score matmuls) must never queue
                # behind PV/filler work on the PE.
                for i in range(len(steps)):
                    emit_sc(i)
                for f in qk_fillers:
                    f()
                for f in v_pre:
                    f()
                # PV + normalize, qb-major so only 2 attps banks are live;
                # the proj units for the last tq chase each norm directly.
                for qb in range(4):
                    with prio_band(BAND_PV):
                        attp = ps_att.tile(
                            [128, HPC, 65], f32, tag="att", name=f"att{tq}_{qb}"
                        )
                        emit_pv_qb(qb, attp)
                        emit_norm(qb, attp)
                    if last:
                        for u in proj_units_qb(tq, qb):
                            u()
                for f in v_post:
                    f()

            # software pipeline: qkv(0) q/k as a prologue; the per-tq
            # attention streams carry the remaining qkv/proj matmuls as
            # banded fillers (q/k gate the next exp stream -> band B; v and
            # PV in band C; all proj work backloaded in band D where the
            # final blocks are the most exp-bound).
            xts0 = emit_qkv(0)
            v_plan = {0: v_units_for(0, xts0)}
            loads, qk_plan = {}, {0: [], 1: [], 2: [], 3: []}
            for t in (1, 2, 3):
                load, xts_t, qk_us = qkv_units(t)
                loads[t - 1] = load
                qk_plan[t - 1] = qk_us
                v_plan[t] = v_units_for(t, xts_t)
            qk_plan[3] = proj_units(0) + proj_units(1) + proj_units(2)
            for tq in range(NTQ):
                if tq in loads:
                    loads[tq]()
                emit_att(
                    tq,
                    qk_plan[tq],
                    v_plan[tq] if tq == 0 else [],
                    v_plan.get(tq + 1, []),
                    last=(tq == NTQ - 1),
                )

    nc.compile()
    return nc


_PERM_LO = np.array([64 * (p // 32) + p % 32 for p in range(128)])
_PERM_HI = _PERM_LO + 32
_F8 = ml_dtypes.float8_e4m3


def _split_fp8(t, a):
    """t (f32) -> (hi, lo) e4m3 pair stored at scale a: hi = e4m3(a*t),
    lo = e4m3(a*(t - hi/a)). Power-of-2 scales keep hi exact vs e4m3(t)."""
    hi = (a * t).astype(_F8)
    lo = (a * t - hi.astype(np.float32)).astype(_F8)
    return hi, lo


def _shard_inputs(x, w_qkv, b_qkv, w_proj, b_proj):
    """Full inputs -> per-core input maps. Core c = (batch b=c//4, group g=c%4).

    q/k columns are permuted so head h's dims land on partitions
    32h:32h+32 split into (lo, hi) chunk slots - the DoubleRow k-tile
    layout the fp8 score matmuls expect. x and wqkv ship as fp8
    residual (hi, lo) pairs at scales 4 and 16 (see build_nc).
    """
    in_maps = []
    xt8s = []
    for b in range(B):
        hi, lo = _split_fp8(np.ascontiguousarray(x[b].T), 4.0)
        xt8s.append(np.ascontiguousarray(np.stack([hi, lo], axis=1)))
    for core in range(NCORES):
        b, g = divmod(core, 4)
        qs = slice(256 * g, 256 * (g + 1))
        ks = slice(C + 256 * g, C + 256 * (g + 1))
        vs = slice(2 * C + 256 * g, 2 * C + 256 * (g + 1))
        wq, wk = w_qkv[:, qs], w_qkv[:, ks]
        wqkv = np.concatenate(
            [wq[:, _PERM_LO], wq[:, _PERM_HI], wk[:, _PERM_LO], wk[:, _PERM_HI],
             w_qkv[:, vs]],
            axis=1,
        )
        whi, wlo = _split_fp8(wqkv, 16.0)
        wqkv8 = np.ascontiguousarray(np.stack([wlo, whi]))  # slot0=lo, slot1=hi
        bq, bk = b_qkv[qs], b_qkv[ks]
        bqk = np.ascontiguousarray(
            np.stack(
                [bq[_PERM_LO], bq[_PERM_HI], bk[_PERM_LO], bk[_PERM_HI]], axis=1
            )
        ).astype(np.float32)
        wproj = np.ascontiguousarray(w_proj[256 * g : 256 * (g + 1), :]).astype(
            ml_dtypes.bfloat16
        )
        in_maps.append(
            {"xt8": xt8s[b], "wqkv8": wqkv8, "bqk": bqk, "wproj": wproj}
        )
    return in_maps


def kernel(x, w_qkv, b_qkv, w_proj, b_proj):
    x = np.asarray(x, dtype=np.float32)
    w_qkv = np.asarray(w_qkv, dtype=np.float32)
    b_qkv = np.asarray(b_qkv, dtype=np.float32)
    w_proj = np.asarray(w_proj, dtype=np.float32)
    b_proj = np.asarray(b_proj, dtype=np.float32)

    if "nc" not in _CACHE:
        _CACHE["nc"] = build_nc()
    nc = _CACHE["nc"]

    in_maps = _shard_inputs(x, w_qkv, b_qkv, w_proj, b_proj)
    res = run_bass_kernel_spmd(nc, in_maps, list(range(NCORES)))
    # host epilogue: sum head-group partials, add folded bias
    b_eff = (b_qkv[2 * C :].astype(np.float64) @ w_proj.astype(np.float64)
             + b_proj).astype(np.float32)
    out = np.empty((B, T, C), dtype=np.float32)
    for b in range(B):
        acc = res.results[4 * b]["out"].astype(np.float32)
        for g in range(1, 4):
            acc = acc + res.results[4 * b + g]["out"].astype(np.float32)
        out[b] = acc + b_eff
    return out



# revision 92
# speedup vs baseline: 1.0006x; 1.0006x over previous
"""Multi-head causal self-attention (B=2, T=2048, C=1024, H=16, D=64) on 8
Trainium2 NeuronCores.

Sharding: data-parallel over batch (2) x tensor-parallel over heads (4 groups
of 4 heads) = 8 shards, no cross-core communication. Host sums the 4 partial
outputs per batch and adds the (folded) bias.

Precision/PE strategy (the PE and Act engines are co-critical):
- qkv projection: fp8e4m3 DoubleRow with residual splitting. x and wqkv ship
  as (hi, lo) e4m3 pairs at power-of-2 scales (x:4, w:16); the A-term
  (hi*hi, contraction 256 per DR matmul) plus B-term (hi*lo + lo*hi cross
  residuals as the two DR k-tiles of one matmul) all carry a uniform 64x
  scale that the psum->sbuf copy divides out (fused mult+bias tensor_scalar).
  Accuracy is bf16-class at ~40% fewer PE cycles; the dropped lo*lo term is
  ~0.1% of signal.
- scores: q,k stored as e4m3 in a head-local layout (head h's dims at
  partitions 32h:32h+32, lo/hi halves as the two DR k-tiles via a host-side
  wqkv column permutation), one DoubleRow matmul per (key-block, head) at
  tile_position (32h, 0) = 0.5 PE cycles/query. Costs ~1.4e-2 rel err
  (gate 2e-2); fp8 for PV/proj fails the gate and stays bf16.
- exp: head-PAIRED - both heads of a pair write the two banks of one
  [128, 2, 512] psum tile, so one Act instruction covers both (halves the
  ~185ns/instr Act access overhead). ~1/4 of pair-exps are computed on the
  DVE with a bf16 Schraudolph (int16 bitcast of x*2^7/ln2 + 16249; max ~4%
  p error, shared bias cancels in the softmax num/den ratio), balancing the
  Act and DVE engines.
- PV keeps the ones-column trick (65th V column accumulates the softmax
  denominator per query partition) and runs qb-major so only 2 attps banks
  are live (PSUM: 2 filler + 4 score-pair + 2 attps banks = 8).
- attT via XBAR DMA transpose (tq 0-2; c-major 3D dest matches the attT
  layout) and PE transpose on the last tq's short tail chains; out =
  attT.T @ wproj in bf16.

Scheduling: explicit priority bands on the tile list-scheduler. Score/exp
chains keep default (lowest=first) priorities so the Act engine's feed is
never queued behind other PE work; band B = q/k filler units for the next
tq (they gate the NEXT exp stream); band C = v units and qb-major PV+norm
(v(tq+1) is emitted after PV(tq) so PV frees pt tiles - which gate the next
tq's exps via pool rotation - before any deferrable work); band D = all
projection, backloaded into the final (most exp-bound) blocks. Weight/x
DMAs land hi-slots first (A-terms open every psum group). The causal mask
zeroes the diagonal blocks' upper triangle in-place on the otherwise-idle
GPSIMD engine; all bias matmuls fold into the host epilogue (softmax rows
sum to 1, so the V bias contributes bv @ w_proj to every output row).
"""

from contextlib import contextmanager

import numpy as np
import ml_dtypes

import concourse.bass as bass
import concourse.mybir as mybir
import concourse.tile as tile
from concourse import bacc
from concourse.bass_utils import run_bass_kernel_spmd

f32 = mybir.dt.float32
bf16 = mybir.dt.bfloat16
f8e4 = mybir.dt.float8e4
DRMODE = mybir.MatmulPerfMode.DoubleRow
AF = mybir.ActivationFunctionType
ALU = mybir.AluOpType

B, T, C, H, D = 2, 2048, 1024, 16, 64
HPC = 4          # heads per core
NCORES = 8
TQ = 512         # query tile of the attention outer loop
NTQ = T // TQ    # 4
NKC = C // 128   # 8 contraction chunks for the qkv projection
NTT = T // 128   # 16 query 128-blocks
SCALE = 1.0 / 8.0  # 1/sqrt(D)

_CACHE = {}


def build_nc():
    nc = bacc.Bacc("TRN2", target_bir_lowering=False, debug=False)

    # x and wqkv ship as fp8 (hi, lo) residual pairs: hi = e4m3(a*t),
    # lo = e4m3(a*(t - hi/a)), with a=4 for x and a=16 for w. Every qkv
    # product term then carries a uniform 64x scale that the psum->sbuf
    # copy divides out, so qkv accuracy is bf16-class at fp8 DR speed.
    xt_d = nc.dram_tensor("xt8", [C, 2, T], f8e4, kind="ExternalInput")
    wqkv_d = nc.dram_tensor("wqkv8", [2, C, 768], f8e4, kind="ExternalInput")
    bqk_d = nc.dram_tensor("bqk", [128, 4], f32, kind="ExternalInput")
    wproj_d = nc.dram_tensor("wproj", [256, C], bf16, kind="ExternalInput")
    out_d = nc.dram_tensor("out", [T, C], bf16, kind="ExternalOutput")

    with tile.TileContext(nc) as tc:
        with (
            tc.tile_pool(name="const", bufs=1) as const,
            tc.tile_pool(name="xts", bufs=2) as xts_pool,
            tc.tile_pool(name="pt", bufs=64) as pt_pool,
            tc.tile_pool(name="atts", bufs=8) as atts_pool,
            tc.tile_pool(name="rec", bufs=8) as rec_pool,
            tc.tile_pool(name="ot", bufs=8) as ot_pool,
            tc.tile_pool(name="ps_mm", bufs=2, space="PSUM") as ps_mm,
            tc.tile_pool(name="ps_sc", bufs=2, space="PSUM") as ps_sc,
            tc.tile_pool(name="ps_att", bufs=2, space="PSUM") as ps_att,
        ):
            # Priority bands: the tile list-scheduler pops ready work by
            # ascending priority. Score/exp chains keep the default
            # (emission-order, lowest) priorities; later bands hold work
            # that must never delay the Act engine's feed chain.
            BAND_QKV, BAND_PV, BAND_PROJ = 1_000_000, 2_000_000, 3_000_000
            _band_next = {}

            @contextmanager
            def prio_band(band):
                saved = tc.cur_priority
                tc.cur_priority = _band_next.get(band, band)
                try:
                    yield
                finally:
                    _band_next[band] = tc.cur_priority
                    tc.cur_priority = saved
            # ---- resident tensors; DMAs chunked so compute starts early ----
            # wqkv slots (host order): 0 = lo, 1 = hi -- B-term k-tile pairs
            # (w_lo, w_hi) and A-term hi slices both slice positively.
            # x slots (host order): 0 = hi, 1 = lo -- B pairs (x_hi, x_lo).
            wqkv_sb = const.tile([128, NKC, 2, 768], f8e4, tag="wqkv")
            wqkv_r = wqkv_d.rearrange("s (o p) n -> p o s n", p=128)
            # hi slots (s=1) first: the A-term hi*hi matmuls open every
            # psum group; lo slots are only needed once B-terms start.
            nc.scalar.dma_start(wqkv_sb[:, 0:2, 1, :], wqkv_r[:, 0:2, 1, :])
            nc.scalar.dma_start(wqkv_sb[:, 2:NKC, 1, :], wqkv_r[:, 2:NKC, 1, :])
            nc.scalar.dma_start(wqkv_sb[:, 0:4, 0, :], wqkv_r[:, 0:4, 0, :])
            nc.scalar.dma_start(wqkv_sb[:, 4:NKC, 0, :], wqkv_r[:, 4:NKC, 0, :])
            bqk_sb = const.tile([128, 4], f32, tag="bqk")
            nc.scalar.dma_start(bqk_sb[:], bqk_d[:, :])
            wproj_sb = const.tile([128, 2, C], bf16, tag="wproj")
            nc.scalar.dma_start(wproj_sb[:], wproj_d.rearrange("(o p) n -> p o n", p=128))

            # qkT chunks (fp8, host-permuted): 0 = q lo-dims (4 heads x 32),
            # 1 = q hi-dims, 2 = k lo, 3 = k hi. Head h occupies partitions
            # 32h:32h+32; its (lo, hi) slots form the two DoubleRow k-tiles.
            qkT_sb = const.tile([128, 4, T], f8e4, tag="qkT")
            # v in PV-rhs layout: [key mod 128, key block, head, 64 vdims + one]
            v_sb = const.tile([128, NTT, HPC, 65], bf16, tag="v")
            nc.vector.memset(v_sb[:, :, :, 64:65], 1.0)
            # attT: chunk hc: partitions = head-dims of heads (2hc, 2hc+1)
            attT_sb = const.tile([128, 2, T], bf16, tag="attT")
            # identity for PE-transpose of the normalized attention
            ident_sb = const.tile([128, 128], bf16, tag="ident")
            nc.vector.memset(ident_sb[:], 1.0)
            nc.gpsimd.affine_select(
                ident_sb[:],
                ident_sb[:],
                pattern=[[1, 128]],
                compare_op=ALU.is_equal,
                fill=0.0,
                base=0,
                channel_multiplier=-1,
            )
            # lower-triangular causal mask (keep j >= p), applied to diagonal
            # blocks with a DVE multiply (lower latency than gpsimd select)
            tri_sb = const.tile([128, 128], bf16, tag="tri")
            nc.vector.memset(tri_sb[:], 1.0)
            nc.gpsimd.affine_select(
                tri_sb[:],
                tri_sb[:],
                pattern=[[1, 128]],
                compare_op=ALU.is_ge,
                fill=0.0,
                base=0,
                channel_multiplier=-1,
            )

            xt_r = xt_d.rearrange("(o p) s t -> p o s t", p=128)


            def qk_chunk_mms(ps, xts, cp, dma_aligned=False):
                """q/k chunk cp into psum ps: A-term (hi*hi, kc-paired DR)
                then B-term (hi*lo + lo*hi cross residuals, one DR per kc).
                All terms carry the uniform 64x host scale. dma_aligned
                orders terms by the kc of their LAST-arriving operand so the
                startup chunk never stalls on a not-yet-landed DMA piece."""
                c0 = 128 * cp

                def a_term(kcp):
                    return (
                        wqkv_sb[:, 2 * kcp : 2 * kcp + 2, 1, c0 : c0 + 128],
                        xts[:, 2 * kcp : 2 * kcp + 2, 0, :],
                        kcp == 0, False,
                    )

                def b_term(kc):
                    return (
                        wqkv_sb[:, kc, :, c0 : c0 + 128],
                        xts[:, kc, :, :],
                        False, kc == NKC - 1,
                    )

                if dma_aligned:
                    return (
                        [a_term(0)]
                        + [a_term(k) for k in range(1, NKC // 2)]
                        + [b_term(k) for k in range(NKC)]
                    )
                return [a_term(k) for k in range(NKC // 2)] + [
                    b_term(k) for k in range(NKC)
                ]

            def run_mms(ps, mms):
                for lhsT, rhs, start, stop in mms:
                    nc.tensor.matmul(
                        ps[:], lhsT=lhsT, rhs=rhs, perf_mode=DRMODE,
                        start=start, stop=stop, skip_group_check=True,
                    )

            def qk_copy(cp, tqs, ps):
                # qkT = psum/64 + bias, converting to the fp8 score layout
                nc.vector.tensor_scalar(
                    qkT_sb[:, cp, tqs], ps[:], 1.0 / 64.0,
                    bqk_sb[:, cp : cp + 1], op0=ALU.mult, op1=ALU.add,
                )

            def emit_qkv(tq):
                """Prologue: q,k chunks only (v units are deferred to band C
                of att(0) - they gate only PV, and the first exp gates Act)."""
                tqs = slice(TQ * tq, TQ * (tq + 1))
                xts = xts_pool.tile([128, NKC, 2, TQ], f8e4, tag="xts")
                nc.sync.dma_start(xts[:, 0:2, 0, :], xt_r[:, 0:2, 0, tqs])
                nc.sync.dma_start(xts[:, 2:NKC, 0, :], xt_r[:, 2:NKC, 0, tqs])
                nc.sync.dma_start(xts[:, 0:4, 1, :], xt_r[:, 0:4, 1, tqs])
                nc.sync.dma_start(xts[:, 4:NKC, 1, :], xt_r[:, 4:NKC, 1, tqs])
                # step-major over pairs of open psum groups to hide DMA ramp
                for cps in ((0, 2), (1, 3)):
                    ps_pair = [
                        ps_mm.tile([128, TQ], f32, tag="mm", name=f"qk{cp}")
                        for cp in cps
                    ]
                    chunk_mms = [qk_chunk_mms(ps_pair[i], xts, cp)
                                 for i, cp in enumerate(cps)]
                    for j in range(len(chunk_mms[0])):
                        for i in range(len(cps)):
                            lhsT, rhs, start, stop = chunk_mms[i][j]
                            nc.tensor.matmul(
                                ps_pair[i][:], lhsT=lhsT, rhs=rhs,
                                perf_mode=DRMODE, start=start, stop=stop,
                                skip_group_check=True,
                            )
                    for i, cp in enumerate(cps):
                        qk_copy(cp, tqs, ps_pair[i])
                return xts

            def v_units_for(tq, xts):
                """v-projection units for t-slice tq, band C: v gates only PV
                of the same tq, so it runs after PV(tq-1) has released its pt
                tiles (which gate the NEXT tq's exp stream)."""
                tqs = slice(TQ * tq, TQ * (tq + 1))

                def v_unit(tt):
                    def emit():
                        with prio_band(BAND_PV):
                            psv = ps_mm.tile([128, TQ], f32, tag="mm")
                            toff = 128 * tt - TQ * tq
                            for kcp in range(NKC // 2):
                                nc.tensor.matmul(
                                    psv[:, 0:256],
                                    lhsT=xts[:, 2 * kcp : 2 * kcp + 2, 0,
                                             toff : toff + 128],
                                    rhs=wqkv_sb[:, 2 * kcp : 2 * kcp + 2, 1,
                                                512:768],
                                    perf_mode=DRMODE,
                                    start=(kcp == 0), stop=False,
                                    skip_group_check=True,
                                )
                            for kc in range(NKC):
                                nc.tensor.matmul(
                                    psv[:, 0:256],
                                    lhsT=xts[:, kc, :, toff : toff + 128],
                                    rhs=wqkv_sb[:, kc, :, 512:768],
                                    perf_mode=DRMODE,
                                    start=False, stop=(kc == NKC - 1),
                                    skip_group_check=True,
                                )
                            nc.vector.tensor_scalar_mul(
                                v_sb[:, tt, :, 0:64], psv[:, 0:256], 1.0 / 64.0
                            )
                    return emit

                return [v_unit(tt) for tt in range(4 * tq, 4 * tq + 4)]

            def qkv_units(tq):
                """q/k projection for t-slice tq as filler closures in band B:
                they gate tq's exp stream, so they run right after the prior
                tq's scores and before any PV."""
                tqs = slice(TQ * tq, TQ * (tq + 1))
                xts = xts_pool.tile([128, NKC, 2, TQ], f8e4, tag="xts")

                def load():
                    for s in range(2):
                        nc.sync.dma_start(xts[:, 0:4, s, :], xt_r[:, 0:4, s, tqs])
                        nc.sync.dma_start(xts[:, 4:NKC, s, :], xt_r[:, 4:NKC, s, tqs])

                def qk_unit(cp):
                    def emit():
                        with prio_band(BAND_QKV):
                            ps = ps_mm.tile([128, TQ], f32, tag="mm")
                            run_mms(ps, qk_chunk_mms(ps, xts, cp))
                            qk_copy(cp, tqs, ps)
                    return emit

                return load, xts, [qk_unit(cp) for cp in range(4)]

            def proj_units_qb(tq, qb):
                tt = 4 * tq + qb
                ot = [None]

                def emit(nt):
                    with prio_band(BAND_PROJ):
                        ts_ = slice(128 * tt, 128 * (tt + 1))
                        ns = slice(512 * nt, 512 * (nt + 1))
                        pso = ps_mm.tile([128, TQ], f32, tag="mm")
                        for hc in range(2):
                            nc.tensor.matmul(
                                pso[:],
                                lhsT=attT_sb[:, hc, ts_],
                                rhs=wproj_sb[:, hc, ns],
                                start=(hc == 0),
                                stop=(hc == 1),
                            )
                        if nt == 0:
                            ot[0] = ot_pool.tile(
                                [128, 2, TQ], bf16, tag="ot", name=f"ot{tt}"
                            )
                        if tq == NTQ - 1:
                            # last block: Act is done with exps by now while
                            # the DVE still drains normalize chains - use Act
                            # for the tail's psum copies, and ship each half
                            # as soon as its copy lands (HWDGE is idle at the
                            # end; a merged DMA would serialize the tail)
                            nc.scalar.copy(ot[0][:, nt, :], pso[:])
                            nc.sync.dma_start(out_d[ts_, ns], ot[0][:, nt, :])
                        else:
                            nc.vector.tensor_copy(ot[0][:, nt, :], pso[:])
                            if nt == 1:
                                # one merged DMA per 128-row block (fewer DMAs
                                # = less serialization on the 1-slot HWDGE)
                                nc.sync.dma_start(out_d[ts_, :], ot[0][:])

                return [lambda: emit(0), lambda: emit(1)]

            def proj_units(tq):
                units = []
                for qb in range(4):
                    units.extend(proj_units_qb(tq, qb))
                return units

            def emit_att(tq, qk_fillers, v_pre, v_post, last=False):
                """Attention for tq: the score/exp stream is emitted at the
                default (highest) priority band so the Act engine is never
                starved; q/k fillers for the next tq go in band B (they gate
                the NEXT tq's exp stream); PV runs qb-major in band C (it
                gates only the projection) with the per-qb normalize chain
                right after its PV chain; v units bracket PV in band C
                (v(tq) before PV(tq), v(tq+1) after, so PV(tq) frees pt
                tiles before any deferrable work); proj units in band D."""
                ntk = 4 * tq + 4
                # steps are (tk, head-pair): both heads' DR score matmuls land
                # in one 2-bank psum tile so a single exp covers them.
                steps = [(tk, hp) for tk in range(ntk) for hp in range(2)]
                pts = {}

                def emit_sc(i):
                    tk, hp = steps[i]
                    d = tk - 4 * tq
                    q0 = 128 * d if d >= 0 else 0
                    w = TQ - q0
                    ks = slice(128 * tk, 128 * (tk + 1))
                    qs = slice(TQ * tq + q0, TQ * (tq + 1))
                    sc = ps_sc.tile([128, 2, TQ], f32, tag="sc")
                    for sl in range(2):
                        h = 2 * hp + sl
                        p0 = 32 * h
                        nc.tensor.matmul(
                            sc[:, sl, 0:w],
                            lhsT=qkT_sb[p0 : p0 + 32, 2:4, ks],
                            rhs=qkT_sb[p0 : p0 + 32, 0:2, qs],
                            perf_mode=DRMODE,
                            tile_position=(p0, 0),
                        )
                    pt = pt_pool.tile([128, 2, TQ], bf16, tag="pt")
                    if (tq < 3 or i < 16) and i % 4 == 1:
                        # Schraudolph exp on the DVE: bf16 bits of exp2(x) ~
                        # round(x*2^7/ln2 + (16256 - C)); C=7 balances the
                        # piecewise-linear error (max ~4%, mean-bias ~0.2%,
                        # and the softmax num/den ratio cancels shared bias).
                        # Offloads ~1/7 of the exp stream from the saturated
                        # Act engine to the half-idle DVE.
                        nc.vector.tensor_scalar(
                            pt[:, :, 0:w].bitcast(mybir.dt.int16),
                            sc[:, :, 0:w],
                            SCALE * 184.6650292,
                            16249.0,
                            op0=ALU.mult,
                            op1=ALU.add,
                        )
                    else:
                        nc.scalar.activation(
                            pt[:, :, 0:w], sc[:, :, 0:w], AF.Exp, scale=SCALE
                        )
                    if d >= 0:
                        # causal mask on the diagonal block: zero the upper
                        # triangle in-place on the (otherwise idle) GPSIMD
                        # engine, freeing the DVE for exp/norm chains
                        for sl in range(2):
                            nc.gpsimd.affine_select(
                                pt[:, sl, 0:128],
                                pt[:, sl, 0:128],
                                pattern=[[1, 128]],
                                compare_op=ALU.is_ge,
                                fill=0.0,
                                base=0,
                                channel_multiplier=-1,
                            )
                    pts[i] = pt

                def emit_pv_qb(qb, attp):
                    for tk in range(4 * tq + qb + 1):
                        d = tk - 4 * tq
                        q0 = 128 * d if d >= 0 else 0
                        qoff = 128 * qb - q0
                        for h in range(HPC):
                            nc.tensor.matmul(
                                attp[:, h, 0:65],
                                lhsT=pts[2 * tk + h // 2][:, h % 2, qoff : qoff + 128],
                                rhs=v_sb[:, tk, h, 0:65],
                                start=(tk == 0 and h == 0),
                                stop=(tk == 4 * tq + qb and h == HPC - 1),
                                skip_group_check=True,
                            )

                def emit_norm(qb, attp):
                    rec = rec_pool.tile([128, HPC], f32, tag="rec")
                    nc.vector.reciprocal_approx_fast(out=rec[:], in_=attp[:, :, 64])
                    att_sb = atts_pool.tile([128, HPC, 64], bf16, tag="atts")
                    nc.vector.tensor_mul(
                        att_sb[:],
                        attp[:, :, 0:64],
                        rec[:, :, None].broadcast_to([128, HPC, 64]),
                    )
                    qslice = slice(TQ * tq + 128 * qb, TQ * tq + 128 * (qb + 1))
                    if tq < NTQ - 1:
                        # XBAR DMA transpose (att [q, hd] -> attT [hd, q]; the
                        # 3D dest maps transposed row r to (chunk r//128,
                        # partition r%128) = exactly the attT layout). Frees
                        # PE/DVE cycles and keeps the ps_att rotation to
                        # attps tiles only, so PV qb-chains overlap; its
                        # ~2.2us latency rides band C/D slack. SP queue: Act's
                        # SEQ must not pay the DMA setup time.
                        nc.sync.dma_start_transpose(
                            attT_sb[:, :, qslice], att_sb[:]
                        )
                    else:
                        # last tq: PE transpose (~53ns) keeps the terminal
                        # norm->proj chain short
                        attTps = ps_att.tile([128, 2, 128], bf16, tag="att")
                        for hc in range(2):
                            nc.tensor.matmul(
                                attTps[:, hc, :],
                                lhsT=att_sb[:, 2 * hc : 2 * hc + 2, :],
                                rhs=ident_sb[:],
                                is_transpose=True,
                                start=(hc == 0),
                                stop=(hc == 1),
                                skip_group_check=True,
                            )
                        nc.vector.tensor_copy(attT_sb[:, :, qslice], attTps[:])

                # score/exp stream at top priority: Act is the near-critical
                # engine, so its feed chain (score matmuls) must never queue
                # behind PV/filler work on the PE.
                for i in range(len(steps)):
                    emit_sc(i)
                for f in qk_fillers:
                    f()
                for f in v_pre:
                    f()
                # PV + normalize, qb-major so only 2 attps banks are live;
                # the proj units for the last tq chase each norm directly.
                for qb in range(4):
                    with prio_band(BAND_PV):
                        attp = ps_att.tile(
                            [128, HPC, 65], f32, tag="att", name=f"att{tq}_{qb}"
                        )
                        emit_pv_qb(qb, attp)
                        emit_norm(qb, attp)
                    if last:
                        for u in proj_units_qb(tq, qb):
                            u()
                for f in v_post:
                    f()

            # software pipeline: qkv(0) q/k as a prologue; the per-tq
            # attention streams carry the remaining qkv/proj matmuls as
            # banded fillers (q/k gate the next exp stream -> band B; v and
            # PV in band C; all proj work backloaded in band D where the
            # final blocks are the most exp-bound).
            xts0 = emit_qkv(0)
            v_plan = {0: v_units_for(0, xts0)}
            loads, qk_plan = {}, {0: [], 1: [], 2: [], 3: []}
            for t in (1, 2, 3):
                load, xts_t, qk_us = qkv_units(t)
                loads[t - 1] = load
                qk_plan[t - 1] = qk_us
                v_plan[t] = v_units_for(t, xts_t)
            qk_plan[3] = proj_units(0) + proj_units(1) + proj_units(2)
            for tq in range(NTQ):
                if tq in loads:
                    loads[tq]()
                emit_att(
                    tq,
                    qk_plan[tq],
                    v_plan[tq] if tq == 0 else [],
                    v_plan.get(tq + 1, []),
                    last=(tq == NTQ - 1),
                )

    nc.compile()
    return nc


_PERM_LO = np.array([64 * (p // 32) + p % 32 for p in range(128)])
_PERM_HI = _PERM_LO + 32
_F8 = ml_dtypes.float8_e4m3


def _split_fp8(t, a):
    """t (f32) -> (hi, lo) e4m3 pair stored at scale a: hi = e4m3(a*t),
    lo = e4m3(a*(t - hi/a)). Power-of-2 scales keep hi exact vs e4m3(t)."""
    hi = (a * t).astype(_F8)
    lo = (a * t - hi.astype(np.float32)).astype(_F8)
    return hi, lo


def _shard_inputs(x, w_qkv, b_qkv, w_proj, b_proj):
    """Full inputs -> per-core input maps. Core c = (batch b=c//4, group g=c%4).

    q/k columns are permuted so head h's dims land on partitions
    32h:32h+32 split into (lo, hi) chunk slots - the DoubleRow k-tile
    layout the fp8 score matmuls expect. x and wqkv ship as fp8
    residual (hi, lo) pairs at scales 4 and 16 (see build_nc).
    """
    in_maps = []
    xt8s = []
    for b in range(B):
        hi, lo = _split_fp8(np.ascontiguousarray(x[b].T), 4.0)
        xt8s.append(np.ascontiguousarray(np.stack([hi, lo], axis=1)))
    for core in range(NCORES):
        b, g = divmod(core, 4)
        qs = slice(256 * g, 256 * (g + 1))
        ks = slice(C + 256 * g, C + 256 * (g + 1))
        vs = slice(2 * C + 256 * g, 2 * C + 256 * (g + 1))
        wq, wk = w_qkv[:, qs], w_qkv[:, ks]
        wqkv = np.concatenate(
            [wq[:, _PERM_LO], wq[:, _PERM_HI], wk[:, _PERM_LO], wk[:, _PERM_HI],
             w_qkv[:, vs]],
            axis=1,
        )
        whi, wlo = _split_fp8(wqkv, 16.0)
        wqkv8 = np.ascontiguousarray(np.stack([wlo, whi]))  # slot0=lo, slot1=hi
        bq, bk = b_qkv[qs], b_qkv[ks]
        bqk = np.ascontiguousarray(
            np.stack(
                [bq[_PERM_LO], bq[_PERM_HI], bk[_PERM_LO], bk[_PERM_HI]], axis=1
            )
        ).astype(np.float32)
        wproj = np.ascontiguousarray(w_proj[256 * g : 256 * (g + 1), :]).astype(
            ml_dtypes.bfloat16
        )
        in_maps.append(
            {"xt8": xt8s[b], "wqkv8": wqkv8, "bqk": bqk, "wproj": wproj}
        )
    return in_maps


def kernel(x, w_qkv, b_qkv, w_proj, b_proj):
    x = np.asarray(x, dtype=np.float32)
    w_qkv = np.asarray(w_qkv, dtype=np.float32)
    b_qkv = np.asarray(b_qkv, dtype=np.float32)
    w_proj = np.asarray(w_proj, dtype=np.float32)
    b_proj = np.asarray(b_proj, dtype=np.float32)

    if "nc" not in _CACHE:
        _CACHE["nc"] = build_nc()
    nc = _CACHE["nc"]

    in_maps = _shard_inputs(x, w_qkv, b_qkv, w_proj, b_proj)
    res = run_bass_kernel_spmd(nc, in_maps, list(range(NCORES)))
    # host epilogue: sum head-group partials, add folded bias
    b_eff = (b_qkv[2 * C :].astype(np.float64) @ w_proj.astype(np.float64)
             + b_proj).astype(np.float32)
    out = np.empty((B, T, C), dtype=np.float32)
    for b in range(B):
        acc = res.results[4 * b]["out"].astype(np.float32)
        for g in range(1, 4):
            acc = acc + res.results[4 * b + g]["out"].astype(np.float32)
        out[b] = acc + b_eff
    return out

